# revision 5
# baseline (speedup 1.0000x reference)
"""CrossAttentionNetwork Bass kernel: bf16 GEMM, unrolled-pipelined loop.

Math (identical to the earlier baseline; see below): the double-softmax
contrastive head collapses algebraically, so the device computes
out = x @ Wq^T + sv with sv = (Wv @ y.sum(axis=1))/1023 host-precomputed
(rel_l2 vs the exact reference ~2e-3, gate 2e-2).  Data-parallel over
batch: 2 batches per core on 8 cores.

Performance structure (all measured on hw via rep-differential timing):
  * bf16 matmuls: ~267 ns per [128x128]x[128,512] call; 48 per problem
    = ~12.8 us PE floor per core.  (fp8e4 DoubleRow was measured at
    ~282 ns/matmul = no win for the 1.5x instruction count of a
    precision-preserving 3-term split, so bf16 stays.)
  * The For_i loop boundary exposes ~9 us of DMA/epilogue latency
    serially per trip: unrolling U problems per trip amortizes it
    (U=1: ~19.7 us, U=8: ~13.3, U=16: ~12.5).
  * Only SP(sync) and ACT(scalar) can issue HWDGE DMAs (gpsimd/Pool
    SWDGE costs +4.6 us/problem - avoid).  Batch 0 I/O rides sync,
    batch 1 rides scalar; inputs for iteration it+k are issued ahead of
    outputs of iteration it where buffering allows.
  * Epilogues (PSUM + sv -> fp16) all on DVE, off the critical path.
  * Warm matmuls only at trip head/tail to hold the PE p-state across
    the loop back edge.
"""

import contextlib

import numpy as np

import concourse.bacc as bacc
import concourse.mybir as mybir
import concourse.tile as tile
from concourse.bass import ts
from concourse.bass_utils import run_bass_kernel_spmd

B, NX, LY = 16, 512, 1024
DIN = 768
DK = DV = 512
N_CORES = 8
BL = B // N_CORES  # 2
DI_CH = DIN // 128  # 6
DK_CH = DK // 128  # 4
INV = 1.0 / (LY - 1.0)

F32 = mybir.dt.float32
F16 = mybir.dt.float16
BF16 = mybir.dt.bfloat16

UNROLL = 16
XBUFS = 3
PREFETCH = True
N_WARM_HEAD = 2
N_WARM_TAIL = 1
COLS_SPLIT = False  # split 512-col matmuls into 2x256 (measured: no win)
OSPLIT = False      # output DMAs in 2 chunks per batch
PSUM8 = True        # batch 1 on its own PSUM banks (acc4-7), warms into acc0


def _build(reps: int = 1):
    """reps = number of full problems executed (16 batches each)."""
    U = min(UNROLL, reps)
    n_for = reps // U
    assert n_for * U == reps, (reps, U)

    nc = bacc.Bacc()
    xt = nc.declare_dram_parameter("xt", [BL, 128, DI_CH, NX], BF16, isOutput=False)
    wq = nc.declare_dram_parameter("wq", [128, DI_CH, DK], BF16, isOutput=False)
    sv = nc.declare_dram_parameter("sv", [128, DK_CH, BL], F32, isOutput=False)
    ot = nc.declare_dram_parameter("ot", [BL, 128, DK_CH, NX], F16, isOutput=True)

    with tile.TileContext(nc) as tc:
        with (
            tc.tile_pool(name="wpool", bufs=1) as wpool,
            tc.tile_pool(name="xpool", bufs=XBUFS) as xpool,
            tc.tile_pool(name="opool", bufs=2) as opool,
            tc.tile_pool(name="acc", bufs=1, space="PSUM") as acc,
            tc.tile_pool(name="zp", bufs=1, space="PSUM") as zp,
        ):
            zpool = acc if PSUM8 else zp
            zsrc = wpool.tile([128, 128], BF16)
            nc.vector.memset(zsrc, 0.0)
            wq_sb = wpool.tile([128, DI_CH, DK], BF16)
            sv_sb = wpool.tile([128, DK_CH, BL], F32)
            if U > 1:
                nc.sync.dma_start(out=wq_sb, in_=wq.ap())
                nc.scalar.dma_start(out=sv_sb, in_=sv.ap())
            else:
                # single-shot: interleave weights and x in need-order so the
                # matmul stream starts as early as possible
                nc.scalar.dma_start(out=sv_sb, in_=sv.ap())

            def warm(n, tag, moving=None):
                ztag = "acc0" if PSUM8 else "zps"
                zps = zpool.tile([128, NX], F32, tag=ztag, name=f"zps_{tag}")
                mv = zsrc if moving is None else moving
                nw = mv.shape[-1]
                for _ in range(n):
                    nc.tensor.matmul(
                        zps[:, 0:nw], zsrc, mv, start=True, stop=True,
                        skip_group_check=True,
                    )

            def xin(it):
                xs = [
                    xpool.tile([128, DI_CH, NX], BF16, tag=f"xt{b}_{it % XBUFS}",
                               name=f"xt{b}_i{it}")
                    for b in range(BL)
                ]
                if it == 0 and U == 1:
                    # single-shot need-order: first matmuls need x[i=0] and
                    # wq[i=0] only; stream the rest behind them
                    nc.sync.dma_start(out=xs[0][:, 0:1, :], in_=xt.ap()[0][:, 0:1, :])
                    nc.sync.dma_start(out=wq_sb[:, 0:3, :], in_=wq.ap()[:, 0:3, :])
                    nc.sync.dma_start(out=xs[0][:, 1:3, :], in_=xt.ap()[0][:, 1:3, :])
                    nc.sync.dma_start(out=wq_sb[:, 3:6, :], in_=wq.ap()[:, 3:6, :])
                    nc.sync.dma_start(out=xs[0][:, 3:6, :], in_=xt.ap()[0][:, 3:6, :])
                elif it == 0:
                    nc.sync.dma_start(out=xs[0][:, 0:1, :], in_=xt.ap()[0][:, 0:1, :])
                    nc.sync.dma_start(out=xs[0][:, 1:3, :], in_=xt.ap()[0][:, 1:3, :])
                    nc.sync.dma_start(out=xs[0][:, 3:6, :], in_=xt.ap()[0][:, 3:6, :])
                else:
                    nc.sync.dma_start(out=xs[0], in_=xt.ap()[0])
                nc.scalar.dma_start(out=xs[1], in_=xt.ap()[1])
                return xs

            def mm(psd, w_ap, x_ap, start, stop):
                if COLS_SPLIT:
                    nc.tensor.matmul(
                        psd[:, 0:256], w_ap, x_ap[:, 0:256],
                        start=start, stop=stop, skip_group_check=True)
                    nc.tensor.matmul(
                        psd[:, 256:512], w_ap, x_ap[:, 256:512],
                        start=start, stop=stop, skip_group_check=True)
                else:
                    nc.tensor.matmul(
                        psd, w_ap, x_ap, start=start, stop=stop,
                        skip_group_check=True)

            rep_ctx = tc.For_i(0, n_for, 1) if n_for > 1 else contextlib.nullcontext()
            with rep_ctx:
                # single-shot (U==1) keeps the PE ramping through the input
                # DMA wait; in the loop two warms suffice (PE never idles)
                warm(10 if U == 1 else N_WARM_HEAD, "head")
                xtiles = {0: xin(0)}
                if PREFETCH and U > 1:
                    xtiles[1] = xin(1)
                for it in range(U):
                    xt_sb = xtiles.pop(it)
                    ot_sb = [
                        opool.tile([128, DK_CH, NX], F16, tag=f"ot{b}_{it % 2}",
                                   name=f"ot{b}_i{it}")
                        for b in range(BL)
                    ]
                    # batch 0: i-outer prefix (overlaps input DMA on the first
                    # trip), then c-outer close with DVE epilogues
                    ps0 = [
                        acc.tile([128, NX], F32, tag=f"acc{c}", name=f"ps0_{c}_i{it}")
                        for c in range(DK_CH)
                    ]
                    for i in range(3):
                        for c in range(DK_CH):
                            mm(ps0[c], wq_sb[:, i, ts(c, 128)], xt_sb[0][:, i, :],
                               start=(i == 0), stop=False)
                    for c in range(DK_CH):
                        for i in range(3, DI_CH):
                            mm(ps0[c], wq_sb[:, i, ts(c, 128)], xt_sb[0][:, i, :],
                               start=False, stop=(i == DI_CH - 1))
                        nc.vector.tensor_scalar_add(
                            ot_sb[0][:, c, :], ps0[c], sv_sb[:, c, 0:1]
                        )
                        if U == 1 and c == 1:
                            nc.sync.dma_start(out=ot.ap()[0][:, 0:2],
                                              in_=ot_sb[0][:, 0:2])
                        elif U == 1 and c == 3:
                            nc.sync.dma_start(out=ot.ap()[0][:, 2:4],
                                              in_=ot_sb[0][:, 2:4])
                    # batch 1: c-outer
                    for c in range(DK_CH):
                        tag1 = f"acc{c + 4}" if PSUM8 else f"acc{c}"
                        ps = acc.tile([128, NX], F32, tag=tag1,
                                      name=f"ps1_{c}_i{it}")
                        for i in range(DI_CH):
                            mm(ps, wq_sb[:, i, ts(c, 128)], xt_sb[1][:, i, :],
                               start=(i == 0), stop=(i == DI_CH - 1))
                        nc.vector.tensor_scalar_add(
                            ot_sb[1][:, c, :], ps, sv_sb[:, c, 1:2]
                        )
                    # inputs for it+2 lead the outputs of it on both queues
                    nxt = it + 2 if PREFETCH else it + 1
                    if nxt < U and nxt not in xtiles:
                        xtiles[nxt] = xin(nxt)
                    if it == U - 1:
                        warm(N_WARM_TAIL, "tail", moving=ot_sb[1][:, 3, :])
                    if U == 1:
                        nc.scalar.dma_start(out=ot.ap()[1][:, 0:3], in_=ot_sb[1][:, 0:3])
                        nc.scalar.dma_start(out=ot.ap()[1][:, 3:4], in_=ot_sb[1][:, 3:4])
                    elif OSPLIT:
                        nc.sync.dma_start(out=ot.ap()[0][:, 0:2], in_=ot_sb[0][:, 0:2])
                        nc.sync.dma_start(out=ot.ap()[0][:, 2:4], in_=ot_sb[0][:, 2:4])
                        nc.scalar.dma_start(out=ot.ap()[1][:, 0:2], in_=ot_sb[1][:, 0:2])
                        nc.scalar.dma_start(out=ot.ap()[1][:, 2:4], in_=ot_sb[1][:, 2:4])
                    else:
                        nc.sync.dma_start(out=ot.ap()[0], in_=ot_sb[0])
                        nc.scalar.dma_start(out=ot.ap()[1], in_=ot_sb[1])

    nc.finalize()
    return nc


_CACHE: dict = {}


def _pack(x, y, Wq, Wk, Wv):
    import ml_dtypes

    bf = ml_dtypes.bfloat16
    xt = np.ascontiguousarray(
        x.reshape(B, NX, DI_CH, 128).transpose(0, 3, 2, 1).astype(bf)
    )
    wqt = np.ascontiguousarray(
        Wq.reshape(DK, DI_CH, 128).transpose(2, 1, 0).astype(bf)
    )
    ysum = y.sum(axis=1, dtype=np.float64)
    svf = (ysum @ Wv.T.astype(np.float64)) * INV
    svt = np.ascontiguousarray(
        svf.reshape(B, DK_CH, 128).transpose(2, 1, 0).astype(np.float32)
    )
    in_maps = []
    for core in range(N_CORES):
        g = slice(core * BL, (core + 1) * BL)
        in_maps.append(
            {
                "xt": xt[g],
                "wq": wqt,
                "sv": np.ascontiguousarray(svt[:, :, g]),
            }
        )
    return in_maps


def _unpack(results):
    out = np.empty((B, NX, DV), dtype=np.float32)
    for core in range(N_CORES):
        o = results[core]["ot"]
        for b in range(BL):
            out[core * BL + b] = (
                o[b].transpose(2, 1, 0).reshape(NX, DV).astype(np.float32)
            )
    return out


def kernel(x, y, Wq, Wk, Wv):
    x = np.asarray(x, dtype=np.float32)
    y = np.asarray(y, dtype=np.float32)
    Wq = np.asarray(Wq, dtype=np.float32)
    Wk = np.asarray(Wk, dtype=np.float32)
    Wv = np.asarray(Wv, dtype=np.float32)
    in_maps = _pack(x, y, Wq, Wk, Wv)
    if "nc" not in _CACHE:
        _CACHE["nc"] = _build()
    res = run_bass_kernel_spmd(_CACHE["nc"], in_maps, core_ids=list(range(N_CORES)))
    return _unpack(res.results)


# revision 6
# speedup vs baseline: 1.0876x; 1.0876x over previous
"""CrossAttentionNetwork Bass kernel: bf16 GEMM, unrolled-pipelined loop.

Math (identical to the earlier baseline; see below): the double-softmax
contrastive head collapses algebraically, so the device computes
out = x @ Wq^T + sv with sv = (Wv @ y.sum(axis=1))/1023 host-precomputed
(rel_l2 vs the exact reference ~2e-3, gate 2e-2).  Data-parallel over
batch: 2 batches per core on 8 cores.

Performance structure (all measured on hw via rep-differential timing):
  * bf16 matmuls: ~267 ns per [128x128]x[128,512] call; 48 per problem
    = ~12.8 us PE floor per core.  (fp8e4 DoubleRow was measured at
    ~282 ns/matmul = no win for the 1.5x instruction count of a
    precision-preserving 3-term split, so bf16 stays.)
  * The For_i loop boundary exposes ~9 us of DMA/epilogue latency
    serially per trip: unrolling U problems per trip amortizes it
    (U=1: ~19.7 us, U=8: ~13.3, U=16: ~12.5).
  * Only SP(sync) and ACT(scalar) can issue HWDGE DMAs (gpsimd/Pool
    SWDGE costs +4.6 us/problem - avoid).  Batch 0 I/O rides sync,
    batch 1 rides scalar; inputs for iteration it+k are issued ahead of
    outputs of iteration it where buffering allows.
  * Epilogues (PSUM + sv -> fp16) all on DVE, off the critical path.
  * Warm matmuls only at trip head/tail to hold the PE p-state across
    the loop back edge.
"""

import contextlib

import numpy as np

import concourse.bacc as bacc
import concourse.mybir as mybir
import concourse.tile as tile
from concourse.bass import ts
from concourse.bass_utils import run_bass_kernel_spmd

B, NX, LY = 16, 512, 1024
DIN = 768
DK = DV = 512
N_CORES = 8
BL = B // N_CORES  # 2
DI_CH = DIN // 128  # 6
DK_CH = DK // 128  # 4
INV = 1.0 / (LY - 1.0)

F32 = mybir.dt.float32
F16 = mybir.dt.float16
BF16 = mybir.dt.bfloat16

UNROLL = 16
XBUFS = 3
PREFETCH = True
N_WARM_HEAD = 2
N_WARM_TAIL = 1
COLS_SPLIT = False  # split 512-col matmuls into 2x256 (measured: no win)
OSPLIT = False      # output DMAs in 2 chunks per batch
PSUM8 = True        # batch 1 on its own PSUM banks (acc4-7), warms into acc0


def _build(reps: int = 1):
    """reps = number of full problems executed (16 batches each)."""
    U = min(UNROLL, reps)
    n_for = reps // U
    assert n_for * U == reps, (reps, U)

    nc = bacc.Bacc()
    xt = nc.declare_dram_parameter("xt", [BL, 128, DI_CH, NX], BF16, isOutput=False)
    wq = nc.declare_dram_parameter("wq", [128, DI_CH, DK], BF16, isOutput=False)
    sv = nc.declare_dram_parameter("sv", [128, DK_CH, BL], F32, isOutput=False)
    ot = nc.declare_dram_parameter("ot", [BL, 128, DK_CH, NX], F16, isOutput=True)

    with tile.TileContext(nc) as tc:
        with (
            tc.tile_pool(name="wpool", bufs=1) as wpool,
            tc.tile_pool(name="xpool", bufs=XBUFS) as xpool,
            tc.tile_pool(name="opool", bufs=2) as opool,
            tc.tile_pool(name="acc", bufs=1, space="PSUM") as acc,
            tc.tile_pool(name="zp", bufs=1, space="PSUM") as zp,
        ):
            zpool = acc if PSUM8 else zp
            zsrc = wpool.tile([128, 128], BF16)
            nc.vector.memset(zsrc, 0.0)
            wq_sb = wpool.tile([128, DI_CH, DK], BF16)
            sv_sb = wpool.tile([128, DK_CH, BL], F32)
            if U > 1:
                nc.sync.dma_start(out=wq_sb, in_=wq.ap())
                nc.scalar.dma_start(out=sv_sb, in_=sv.ap())
            else:
                # single-shot: interleave weights and x in need-order so the
                # matmul stream starts as early as possible
                nc.scalar.dma_start(out=sv_sb, in_=sv.ap())

            def warm(n, tag, moving=None):
                ztag = "acc0" if PSUM8 else "zps"
                zps = zpool.tile([128, NX], F32, tag=ztag, name=f"zps_{tag}")
                mv = zsrc if moving is None else moving
                nw = mv.shape[-1]
                for _ in range(n):
                    nc.tensor.matmul(
                        zps[:, 0:nw], zsrc, mv, start=True, stop=True,
                        skip_group_check=True,
                    )

            def xin(it):
                xs = [
                    xpool.tile([128, DI_CH, NX], BF16, tag=f"xt{b}_{it % XBUFS}",
                               name=f"xt{b}_i{it}")
                    for b in range(BL)
                ]
                if it == 0 and U == 1:
                    # single-shot need-order: first matmuls need x[i=0] and
                    # wq[i=0] only; stream the rest behind them
                    nc.sync.dma_start(out=xs[0][:, 0:1, :], in_=xt.ap()[0][:, 0:1, :])
                    nc.sync.dma_start(out=wq_sb[:, 0:3, :], in_=wq.ap()[:, 0:3, :])
                    nc.sync.dma_start(out=xs[0][:, 1:3, :], in_=xt.ap()[0][:, 1:3, :])
                    nc.sync.dma_start(out=wq_sb[:, 3:6, :], in_=wq.ap()[:, 3:6, :])
                    nc.sync.dma_start(out=xs[0][:, 3:6, :], in_=xt.ap()[0][:, 3:6, :])
                elif it == 0:
                    nc.sync.dma_start(out=xs[0][:, 0:1, :], in_=xt.ap()[0][:, 0:1, :])
                    nc.sync.dma_start(out=xs[0][:, 1:3, :], in_=xt.ap()[0][:, 1:3, :])
                    nc.sync.dma_start(out=xs[0][:, 3:6, :], in_=xt.ap()[0][:, 3:6, :])
                else:
                    nc.sync.dma_start(out=xs[0], in_=xt.ap()[0])
                nc.scalar.dma_start(out=xs[1], in_=xt.ap()[1])
                return xs

            def mm(psd, w_ap, x_ap, start, stop):
                if COLS_SPLIT:
                    nc.tensor.matmul(
                        psd[:, 0:256], w_ap, x_ap[:, 0:256],
                        start=start, stop=stop, skip_group_check=True)
                    nc.tensor.matmul(
                        psd[:, 256:512], w_ap, x_ap[:, 256:512],
                        start=start, stop=stop, skip_group_check=True)
                else:
                    nc.tensor.matmul(
                        psd, w_ap, x_ap, start=start, stop=stop,
                        skip_group_check=True)

            rep_ctx = tc.For_i(0, n_for, 1) if n_for > 1 else contextlib.nullcontext()
            with rep_ctx:
                # single-shot (U==1) keeps the PE ramping through the input
                # DMA wait; in the loop two warms suffice (PE never idles)
                warm(10 if U == 1 else N_WARM_HEAD, "head")
                xtiles = {0: xin(0)}
                if PREFETCH and U > 1:
                    xtiles[1] = xin(1)
                for it in range(U):
                    xt_sb = xtiles.pop(it)
                    ot_sb = [
                        opool.tile([128, DK_CH, NX], F16, tag=f"ot{b}_{it % 2}",
                                   name=f"ot{b}_i{it}")
                        for b in range(BL)
                    ]
                    # batch 0: i-outer prefix (overlaps input DMA on the first
                    # trip), then c-outer close with DVE epilogues
                    ps0 = [
                        acc.tile([128, NX], F32, tag=f"acc{c}", name=f"ps0_{c}_i{it}")
                        for c in range(DK_CH)
                    ]
                    for i in range(3):
                        for c in range(DK_CH):
                            mm(ps0[c], wq_sb[:, i, ts(c, 128)], xt_sb[0][:, i, :],
                               start=(i == 0), stop=False)
                    for c in range(DK_CH):
                        for i in range(3, DI_CH):
                            mm(ps0[c], wq_sb[:, i, ts(c, 128)], xt_sb[0][:, i, :],
                               start=False, stop=(i == DI_CH - 1))
                        nc.vector.tensor_scalar_add(
                            ot_sb[0][:, c, :], ps0[c], sv_sb[:, c, 0:1]
                        )
                        if U == 1 and c == 1:
                            nc.sync.dma_start(out=ot.ap()[0][:, 0:2],
                                              in_=ot_sb[0][:, 0:2])
                        elif U == 1 and c == 3:
                            nc.sync.dma_start(out=ot.ap()[0][:, 2:4],
                                              in_=ot_sb[0][:, 2:4])
                    # batch 1: c-outer
                    for c in range(DK_CH):
                        tag1 = f"acc{c + 4}" if PSUM8 else f"acc{c}"
                        ps = acc.tile([128, NX], F32, tag=tag1,
                                      name=f"ps1_{c}_i{it}")
                        for i in range(DI_CH):
                            mm(ps, wq_sb[:, i, ts(c, 128)], xt_sb[1][:, i, :],
                               start=(i == 0), stop=(i == DI_CH - 1))
                        nc.vector.tensor_scalar_add(
                            ot_sb[1][:, c, :], ps, sv_sb[:, c, 1:2]
                        )
                    # inputs for it+2 lead the outputs of it on both queues
                    nxt = it + 2 if PREFETCH else it + 1
                    if nxt < U and nxt not in xtiles:
                        xtiles[nxt] = xin(nxt)
                    if it == U - 1:
                        warm(N_WARM_TAIL, "tail", moving=ot_sb[1][:, 3, :])
                    if U == 1:
                        nc.scalar.dma_start(out=ot.ap()[1][:, 0:3], in_=ot_sb[1][:, 0:3])
                        nc.scalar.dma_start(out=ot.ap()[1][:, 3:4], in_=ot_sb[1][:, 3:4])
                    elif it == U - 1:
                        # last iteration of the trip: small final DMAs spread
                        # over both queues shorten the tail exposed at the
                        # For_i back edge
                        nc.sync.dma_start(out=ot.ap()[0][:, 0:3], in_=ot_sb[0][:, 0:3])
                        nc.scalar.dma_start(out=ot.ap()[1][:, 0:3], in_=ot_sb[1][:, 0:3])
                        nc.sync.dma_start(out=ot.ap()[0][:, 3:4], in_=ot_sb[0][:, 3:4])
                        nc.scalar.dma_start(out=ot.ap()[1][:, 3:4], in_=ot_sb[1][:, 3:4])
                    elif OSPLIT:
                        nc.sync.dma_start(out=ot.ap()[0][:, 0:2], in_=ot_sb[0][:, 0:2])
                        nc.sync.dma_start(out=ot.ap()[0][:, 2:4], in_=ot_sb[0][:, 2:4])
                        nc.scalar.dma_start(out=ot.ap()[1][:, 0:2], in_=ot_sb[1][:, 0:2])
                        nc.scalar.dma_start(out=ot.ap()[1][:, 2:4], in_=ot_sb[1][:, 2:4])
                    else:
                        nc.sync.dma_start(out=ot.ap()[0], in_=ot_sb[0])
                        nc.scalar.dma_start(out=ot.ap()[1], in_=ot_sb[1])

    nc.finalize()
    return nc


_CACHE: dict = {}


def _pack(x, y, Wq, Wk, Wv):
    import ml_dtypes

    bf = ml_dtypes.bfloat16
    xt = np.ascontiguousarray(
        x.reshape(B, NX, DI_CH, 128).transpose(0, 3, 2, 1).astype(bf)
    )
    wqt = np.ascontiguousarray(
        Wq.reshape(DK, DI_CH, 128).transpose(2, 1, 0).astype(bf)
    )
    ysum = y.sum(axis=1, dtype=np.float64)
    svf = (ysum @ Wv.T.astype(np.float64)) * INV
    svt = np.ascontiguousarray(
        svf.reshape(B, DK_CH, 128).transpose(2, 1, 0).astype(np.float32)
    )
    in_maps = []
    for core in range(N_CORES):
        g = slice(core * BL, (core + 1) * BL)
        in_maps.append(
            {
                "xt": xt[g],
                "wq": wqt,
                "sv": np.ascontiguousarray(svt[:, :, g]),
            }
        )
    return in_maps


def _unpack(results):
    out = np.empty((B, NX, DV), dtype=np.float32)
    for core in range(N_CORES):
        o = results[core]["ot"]
        for b in range(BL):
            out[core * BL + b] = (
                o[b].transpose(2, 1, 0).reshape(NX, DV).astype(np.float32)
            )
    return out


def kernel(x, y, Wq, Wk, Wv):
    x = np.asarray(x, dtype=np.float32)
    y = np.asarray(y, dtype=np.float32)
    Wq = np.asarray(Wq, dtype=np.float32)
    Wk = np.asarray(Wk, dtype=np.float32)
    Wv = np.asarray(Wv, dtype=np.float32)
    in_maps = _pack(x, y, Wq, Wk, Wv)
    if "nc" not in _CACHE:
        _CACHE["nc"] = _build()
    res = run_bass_kernel_spmd(_CACHE["nc"], in_maps, core_ids=list(range(N_CORES)))
    return _unpack(res.results)
